# revision 35
# baseline (speedup 1.0000x reference)
"""Trainium2 Bass kernel for nn_AetheriusCoreBlock (8-core SPMD).

Design:
  - Host fuses xi = x@Win+bin into the downstream S6 weights (WbF = Win@WB,
    WcF = Win@WC, WdF = Win@Wd; exact algebra), so the device works straight
    from x^T (full, bf16, SBUF-resident).
  - Channel-sharded S6 (128 of 1024 state channels/core): fused WB/WC
    projections (n-major columns, quarter-blocked single-read streaming),
    selective scan via tensor_tensor_scan (HW prefix scan), then an AllToAll
    routes y back to token shards and y@Wout runs token-locally.
  - Token-sharded front (128 tokens/core): GLCM branch (rotary channels
    de-interleaved via host-permuted Wp, depthwise conv as 7 fused
    shifted-window ops, PE-transpose + strided write un-permutes), GFCU gates.
  - MoE: expert-sharded (2 experts/core). Routing logits in f32 (top-2
    selection must match the f32 reference), routing weights exchanged with
    AllToAll; per expert the selected tokens are compacted with sparse_gather
    and gathered with indirect_copy (capacity CAP=192, host fallback on
    overflow); expert weights in fp8(e4m3, x64 scale) to halve DMA; outputs
    returned unweighted and combined on host.

kernel(**inputs) takes FULL inputs (as from setup_inputs) and returns the
FULL [2, 512, 1024] float32 output.
"""

import sys
import numpy as np

sys.path.insert(0, "/opt/trn_rl_repo")

import ml_dtypes

bf16 = ml_dtypes.bfloat16
fp8np = ml_dtypes.float8_e4m3

from concourse import bass, bacc, mybir, tile  # noqa: E402
from concourse import bass_utils  # noqa: E402
from concourse.masks import make_identity  # noqa: E402

F32 = mybir.dt.float32
BF16 = mybir.dt.bfloat16
FP8 = mybir.dt.float8e4
U16 = mybir.dt.uint16
U32 = mybir.dt.uint32
I32 = mybir.dt.int32
ALU = mybir.AluOpType
ACTF = mybir.ActivationFunctionType
AX = mybir.AxisListType

NC_ = 8
B_, T_, D_ = 2, 512, 1024
S_, N_, E_ = 1024, 16, 16
K7 = 7
TOK = B_ * T_
TPC = TOK // NC_     # 128 tokens/core
SPC = S_ // NC_      # 128 state channels/core
CAP = 192            # token capacity per expert (mean 128, +6 sigma)
HALO = 3
THW = TPC + 2 * HALO  # 134
EPS = 1e-8
MOE_WSCALE = 64.0

DEBUG = False
REPEAT = 1           # timing only: emit the body N times in one program


def bcast_ap(t, n_part=128):
    ap = t.ap()[None, :]
    ap.ap[0] = [0, n_part]
    return ap


def build_program(repeat=None):
    repeat = REPEAT if repeat is None else repeat
    nc = bacc.Bacc("TRN2", target_bir_lowering=False, debug=False,
                   num_devices=NC_)

    def inp(name, shape, dt=F32):
        return nc.dram_tensor(name, list(shape), dt, kind="ExternalInput")

    def outp(name, shape, dt=F32):
        return nc.dram_tensor(name, list(shape), dt, kind="ExternalOutput")

    xt_halo = inp("xt_halo", [D_, THW], BF16)
    xt_full = inp("xt_full", [D_, TOK], BF16)
    x_tm = inp("x_tm", [TPC, D_])
    sin_t = inp("sin_t", [D_ // 2, THW])
    cos_t = inp("cos_t", [D_ // 2, THW])
    win_u = inp("win_u", [D_, SPC], BF16)
    bin_u = inp("bin_u", [128, 1])
    wp_blk = inp("wp_blk", [8, 8, 128, 256], BF16)
    bp_h = inp("bp_h", [128, 16])
    wdw_h = inp("wdw_h", [128, 8 * K7])
    bdw_h = inp("bdw_h", [128, 8])
    glcm_g = inp("glcm_g", [D_])
    gfw1 = inp("gfw1", [D_, D_], BF16)
    gfw2 = inp("gfw2", [D_, D_], BF16)
    gfb1 = inp("gfb1", [D_])
    gfb2 = inp("gfb2", [D_])
    gf_g = inp("gf_g", [D_])
    wdf = inp("wdf", [D_, SPC], BF16)
    bd_h = inp("bd_h", [128, 1])
    wbf = inp("wbf", [4, 8, 128, 512], BF16)
    wcf = inp("wcf", [4, 8, 128, 512], BF16)
    bb_h = inp("bb_h", [128, 16])
    bc_h = inp("bc_h", [128, 16])
    a_mat = inp("a_mat", [128, 16])
    wout = inp("wout", [S_, D_], BF16)
    bout = inp("bout", [D_])
    s6g = inp("s6g", [D_])
    wg = inp("wg", [D_, E_])
    w1 = inp("w1", [2, 8, 8, 128, 512], BF16)
    b1_h = inp("b1_h", [2, 128, 32])
    w2 = inp("w2", [2, 32, 128, D_], BF16)

    h_out = outp("h_out", [TPC, D_])
    w_out = outp("w_out", [TPC, E_])
    ids_out = outp("ids_out", [2, 16, CAP // 16])
    nf_out = outp("nf_out", [2, 1], U32)
    e2_out = outp("e2_out", [2, CAP, D_])

    rg = [list(range(NC_))]

    with tile.TileContext(nc) as tc:
        with (
            tc.tile_pool(name="consts", bufs=1) as consts,
            tc.tile_pool(name="per", bufs=1) as per,
            tc.tile_pool(name="wstr", bufs=1) as wstr,
            tc.tile_pool(name="work", bufs=1) as work,
            tc.tile_pool(name="psA", bufs=3, space="PSUM") as psA,
            tc.tile_pool(name="psB", bufs=2, space="PSUM") as psB,
            tc.tile_pool(name="psC", bufs=2, space="PSUM") as psC,
            tc.tile_pool(name="dram", bufs=1, space="DRAM") as dram,
        ):
            ident = consts.tile([128, 128], F32, tag="ident")
            make_identity(nc, ident[:])
            eps_sb = consts.tile([128, 1], F32, tag="eps_sb")
            nc.vector.memset(eps_sb[:], EPS)

            def cload(name, src_ap, shape, dt=F32):
                t = consts.tile(list(shape), dt, tag=name, name=name)
                eng = nc.gpsimd if (dt == BF16 and src_ap.dtype == F32) \
                    else nc.sync
                eng.dma_start(t[:], src_ap)
                return t

            binu_sb = cload("binu_sb", bin_u.ap(), [128, 1])
            bp_sb = cload("bp_sb", bp_h.ap(), [128, 16])
            bd_sb = cload("bd_sb", bd_h.ap(), [128, 1])
            bb_sb = cload("bb_sb", bb_h.ap(), [128, 16])
            bc_sb = cload("bc_sb", bc_h.ap(), [128, 16])
            a_sb = cload("a_sb", a_mat.ap(), [128, 16])
            wdw_sb = cload("wdw_sb", wdw_h.ap(), [128, 8 * K7])
            bdw_sb = cload("bdw_sb", bdw_h.ap(), [128, 8])
            b1_sb = [cload(f"b1_sb{e}", b1_h.ap()[e], [128, 32])
                     for e in (0, 1)]
            glcmg_bc = cload("glcmg_bc", bcast_ap(glcm_g), [128, D_], BF16)
            s6g_bc = cload("s6g_bc", bcast_ap(s6g), [128, D_], BF16)
            gfg_bc = cload("gfg_bc", bcast_ap(gf_g), [128, D_], BF16)
            bout_bc = cload("bout_bc", bcast_ap(bout), [128, D_], BF16)
            gfb1_bc = cload("gfb1_bc", bcast_ap(gfb1), [128, D_], BF16)
            gfb2_bc = cload("gfb2_bc", bcast_ap(gfb2), [128, D_], BF16)
            sin_sb = [cload(f"sin{i}", sin_t.ap()[i * 128:(i + 1) * 128, :],
                            [128, THW]) for i in range(4)]
            cos_sb = [cload(f"cos{i}", cos_t.ap()[i * 128:(i + 1) * 128, :],
                            [128, THW]) for i in range(4)]
            wg_sb = [cload(f"wg_sb{k}", wg.ap()[k * 128:(k + 1) * 128, :],
                           [128, E_], F32) for k in range(8)]

            xt = []
            for k in range(8):
                t = per.tile([128, THW], BF16, tag=f"xt{k}")
                nc.sync.dma_start(t[:], xt_halo.ap()[k * 128:(k + 1) * 128, :])
                xt.append(t)
            xt_c = [t[:, HALO:HALO + TPC] for t in xt]

            xtf = []
            for k in range(8):
                t = per.tile([128, TOK], BF16, tag=f"xtf{k}")
                nc.sync.dma_start(t[:], xt_full.ap()[k * 128:(k + 1) * 128, :])
                xtf.append(t)

            x_sb = per.tile([TPC, D_], F32, tag="x_sb")
            nc.sync.dma_start(x_sb[:], x_tm.ap())


            iota_i = per.tile([16, 64], I32, tag="iota_i")
            nc.gpsimd.iota(iota_i[:], pattern=[[16, 64]], base=0,
                           channel_multiplier=1)
            iota_f1 = per.tile([16, 64], F32, tag="iota_f1")
            nc.vector.tensor_copy(out=iota_f1[:], in_=iota_i[:])
            nc.vector.tensor_scalar_add(iota_f1[:], iota_f1[:], 1.0)

            def emit_body():
                ya_in = dram.tile([NC_ * 128, TPC], BF16, name="ya_in")
                ya_out = dram.tile([NC_ * 128, TPC], BF16, name="ya_out")
                agin_h = dram.tile([D_, TPC], BF16, name="agin_h")
                ag_h = dram.tile([NC_ * D_, TPC], BF16,
                                 addr_space="Shared", name="ag_h")
                a2a_in = dram.tile([2 * NC_, TPC], F32, name="a2a_in")
                a2a_out = dram.tile([2 * NC_, TPC], F32, name="a2a_out")
                # ---- delta / u (all tokens, f32) ----
                delta = per.tile([128, TOK], F32, tag="delta")
                u_sb = per.tile([128, TOK], F32, tag="u_sb")
                for wi, (wsrc, dst, bias, is_sp) in enumerate((
                        (wdf, delta, bd_sb, True),
                        (win_u, u_sb, binu_sb, False))):
                    wt_l = []
                    for k in range(8):
                        t = wstr.tile([128, SPC], BF16, tag="lhs128", bufs=16,
                                      name=f"du_w{wi}_{k}")
                        nc.sync.dma_start(t[:],
                                          wsrc.ap()[k * 128:(k + 1) * 128, :])
                        wt_l.append(t)
                    for hf in range(2):
                        ps = psB.tile([128, 512], F32, tag="pmid")
                        for k in range(8):
                            nc.tensor.matmul(
                                ps[:], wt_l[k][:],
                                xtf[k][:, hf * 512:(hf + 1) * 512],
                                start=(k == 0), stop=(k == 7))
                        if is_sp:
                            spt = work.tile([128, 512], F32, tag="gftmp",
                                            bufs=2, name=f"spt{hf}")
                            nc.scalar.activation(spt[:], ps[:], ACTF.Exp,
                                                 bias=bias[:])
                            nc.scalar.activation(
                                dst[:, hf * 512:(hf + 1) * 512], spt[:],
                                ACTF.Ln, bias=1.0)
                        else:
                            nc.scalar.activation(
                                dst[:, hf * 512:(hf + 1) * 512], ps[:],
                                ACTF.Identity, bias=bias[:])
                du = per.tile([128, TOK], F32, tag="du")
                nc.vector.tensor_mul(du[:], delta[:], u_sb[:])

                # ---- S6: fused WB/WC + scan, per (nq, n4, b) ----
                y_sb = per.tile([128, TOK], F32, tag="y_sb")
                for nq in range(4):
                    wbq, wcq = [], []
                    for k in range(8):
                        t = wstr.tile([128, 512], BF16, tag="wq", bufs=16,
                                      name=f"wbq{nq}_{k}")
                        nc.sync.dma_start(t[:], wbf.ap()[nq, k])
                        wbq.append(t)
                        t2 = wstr.tile([128, 512], BF16, tag="wq", bufs=16,
                                       name=f"wcq{nq}_{k}")
                        nc.sync.dma_start(t2[:], wcf.ap()[nq, k])
                        wcq.append(t2)
                    for n4 in range(4):
                        n = nq * 4 + n4
                        for b in range(B_):
                            sl = slice(b * T_, (b + 1) * T_)
                            ps_b = psA.tile([128, T_], F32, tag="pnb")
                            for k in range(8):
                                nc.tensor.matmul(
                                    ps_b[:],
                                    wbq[k][:, n4 * 128:(n4 + 1) * 128],
                                    xtf[k][:, sl], start=(k == 0),
                                    stop=(k == 7))
                            a_t = work.tile([128, T_], F32, tag="a_t",
                                            bufs=2)
                            nc.scalar.activation(a_t[:], delta[:, sl],
                                                 ACTF.Exp,
                                                 scale=a_sb[:, n:n + 1])
                            dbu = work.tile([128, T_], F32, tag="dbu",
                                            bufs=2)
                            nc.vector.scalar_tensor_tensor(
                                dbu[:], ps_b[:], bb_sb[:, n:n + 1],
                                du[:, sl], op0=ALU.add, op1=ALU.mult)
                            H_t = work.tile([128, T_], F32, tag="H_t",
                                            bufs=2)
                            nc.vector.tensor_tensor_scan(
                                H_t[:], a_t[:], dbu[:], initial=0.0,
                                op0=ALU.mult, op1=ALU.add)
                            ps_c = psA.tile([128, T_], F32, tag="pnb")
                            for k in range(8):
                                nc.tensor.matmul(
                                    ps_c[:],
                                    wcq[k][:, n4 * 128:(n4 + 1) * 128],
                                    xtf[k][:, sl], start=(k == 0),
                                    stop=(k == 7))
                            if n == 0:
                                nc.vector.scalar_tensor_tensor(
                                    y_sb[:, sl], ps_c[:],
                                    bc_sb[:, n:n + 1], H_t[:],
                                    op0=ALU.add, op1=ALU.mult)
                            else:
                                yt = work.tile([128, T_], F32, tag="yt",
                                               bufs=2)
                                nc.vector.scalar_tensor_tensor(
                                    yt[:], ps_c[:], bc_sb[:, n:n + 1],
                                    H_t[:], op0=ALU.add, op1=ALU.mult)
                                nc.vector.tensor_add(y_sb[:, sl], yt[:],
                                                     y_sb[:, sl])

                # ---- y AllToAll (token-shard routing) + local Wout ----
                y_bf = per.tile([128, TOK], BF16, tag="y_bf")
                nc.scalar.copy(y_bf[:], y_sb[:])
                for j in range(8):
                    nc.sync.dma_start(
                        ya_in[:].rearrange("(c p) t -> c p t", c=8)[j],
                        y_bf[:, j * 128:(j + 1) * 128])
                nc.gpsimd.collective_compute(
                    "AllToAll", ALU.bypass, replica_groups=rg,
                    ins=[ya_in[:].opt()], outs=[ya_out[:].opt()])
                ya_v = ya_out[:].rearrange("(r p) t -> r p t", r=8)
                yfull = []
                for k in range(8):
                    t = per.tile([128, TPC], BF16, tag=f"yfull{k}",
                                 name=f"yfull{k}")
                    nc.sync.dma_start(t[:], ya_v[k])
                    yfull.append(t)
                s6v = per.tile([TPC, D_], F32, tag="s6v")
                pso = [psB.tile([128, 512], F32, tag="pmid",
                                name=f"pso{hf}") for hf in range(2)]
                for k in range(8):
                    wt = wstr.tile([128, D_], BF16, tag="wmed", bufs=3,
                                   name=f"wout_{k}")
                    nc.sync.dma_start(wt[:],
                                      wout.ap()[k * 128:(k + 1) * 128, :])
                    for hf in range(2):
                        nc.tensor.matmul(pso[hf][:], yfull[k][:],
                                         wt[:, hf * 512:(hf + 1) * 512],
                                         start=(k == 0), stop=(k == 7))
                for hf in range(2):
                    nc.vector.tensor_copy(
                        out=s6v[:, hf * 512:(hf + 1) * 512], in_=pso[hf][:])

                # ---- GLCM ----
                wp_cache = {}

                def p_psum(blk, half):
                    ps = psC.tile([128, THW], F32, tag="psmall")
                    for k in range(8):
                        if half == 0:
                            wt = wstr.tile([128, 256], BF16, tag="wp_s",
                                           bufs=16, name=f"wp{blk}_{k}")
                            nc.sync.dma_start(wt[:], wp_blk.ap()[blk, k])
                            wp_cache[(blk, k)] = wt
                        wt = wp_cache[(blk, k)]
                        nc.tensor.matmul(
                            ps[:], wt[:, half * 128:(half + 1) * 128],
                            xt[k][:], start=(k == 0), stop=(k == 7))
                    return ps

                xr = [per.tile([128, THW], F32, tag=f"xr{i}", name=f"xr{i}")
                      for i in range(8)]
                for i in range(4):
                    ps_e = p_psum(i, 0)
                    ps_o = p_psum(i, 1)
                    t1 = work.tile([128, THW], F32, tag="rot1", bufs=2)
                    t2 = work.tile([128, THW], F32, tag="rot2", bufs=2)
                    nc.vector.scalar_tensor_tensor(
                        t1[:], ps_e[:], bp_sb[:, i:i + 1], cos_sb[i][:],
                        op0=ALU.add, op1=ALU.mult)
                    nc.vector.scalar_tensor_tensor(
                        t2[:], ps_o[:], bp_sb[:, i + 4:i + 5], sin_sb[i][:],
                        op0=ALU.add, op1=ALU.mult)
                    nc.vector.tensor_sub(xr[i][:], t1[:], t2[:])
                    nc.vector.scalar_tensor_tensor(
                        t1[:], ps_e[:], bp_sb[:, i:i + 1], sin_sb[i][:],
                        op0=ALU.add, op1=ALU.mult)
                    nc.vector.scalar_tensor_tensor(
                        t2[:], ps_o[:], bp_sb[:, i + 4:i + 5], cos_sb[i][:],
                        op0=ALU.add, op1=ALU.mult)
                    nc.vector.tensor_add(xr[i + 4][:], t1[:], t2[:])

                glcm_tm = per.tile([TPC, D_], F32, tag="glcm_tm")
                for dt_i in range(8):
                    acc = work.tile([128, TPC], F32, tag="convacc", bufs=2)
                    nc.vector.tensor_scalar_mul(
                        acc[:], xr[dt_i][:, 0:TPC],
                        wdw_sb[:, dt_i * K7:dt_i * K7 + 1])
                    for k in range(1, K7):
                        nc.vector.scalar_tensor_tensor(
                            acc[:], xr[dt_i][:, k:k + TPC],
                            wdw_sb[:, dt_i * K7 + k:dt_i * K7 + k + 1],
                            acc[:], op0=ALU.mult, op1=ALU.add)
                    sil = work.tile([128, TPC], F32, tag="convsil", bufs=2)
                    nc.scalar.activation(sil[:], acc[:], ACTF.Silu,
                                         bias=bdw_sb[:, dt_i:dt_i + 1])
                    ps_xb = p_psum(4 + dt_i // 2, dt_i % 2)
                    sg = work.tile([128, TPC], F32, tag="convsg", bufs=2)
                    nc.scalar.activation(sg[:], ps_xb[:, HALO:HALO + TPC],
                                         ACTF.Sigmoid,
                                         bias=bp_sb[:, 8 + dt_i:9 + dt_i])
                    gated = work.tile([128, TPC], F32, tag="gated", bufs=2)
                    nc.vector.tensor_mul(gated[:], sil[:], sg[:])
                    pst = psB.tile([128, 128], F32, tag="ptp", bufs=1)
                    nc.tensor.transpose(pst[:], gated[:], ident[:])
                    half = dt_i % 4
                    even = dt_i < 4
                    view = glcm_tm[:].rearrange("p (c two) -> p c two",
                                                two=2)
                    dst = view[:, half * 128:(half + 1) * 128,
                               0 if even else 1]
                    nc.vector.tensor_copy(out=dst, in_=pst[:])
                sq = work.tile([TPC, D_], F32, tag="scratch")
                ssum = work.tile([TPC, 1], F32, tag="ssum", bufs=2)
                nc.scalar.activation(sq[:], glcm_tm[:], ACTF.Square,
                                     accum_out=ssum[:])
                rr = work.tile([TPC, 1], F32, tag="rr", bufs=2)
                nc.scalar.activation(rr[:], ssum[:], ACTF.Sqrt,
                                     scale=1.0 / D_, bias=eps_sb[:])
                rinv = work.tile([TPC, 1], F32, tag="rinv", bufs=2)
                nc.vector.reciprocal(rinv[:], rr[:])
                nc.vector.scalar_tensor_tensor(
                    glcm_tm[:], glcm_tm[:], rinv[:], glcmg_bc[:],
                    op0=ALU.mult, op1=ALU.mult)

                # ---- gf gates ----
                g1 = per.tile([TPC, D_], F32, tag="g1")
                g2 = per.tile([TPC, D_], F32, tag="g2")
                for gi, (gt, wgf, gb) in enumerate(
                        ((g1, gfw1, gfb1_bc), (g2, gfw2, gfb2_bc))):
                    pss = [psB.tile([128, 512], F32, tag="pmid",
                                    name=f"gps{gi}_{hf}")
                           for hf in range(2)]
                    for k in range(8):
                        wt = wstr.tile([128, D_], BF16, tag="wmed", bufs=3,
                                       name=f"gf{gi}_{k}")
                        nc.sync.dma_start(
                            wt[:], wgf.ap()[k * 128:(k + 1) * 128, :])
                        for hf in range(2):
                            nc.tensor.matmul(
                                pss[hf][:], xt_c[k],
                                wt[:, hf * 512:(hf + 1) * 512],
                                start=(k == 0), stop=(k == 7))
                    for hf in range(2):
                        tmp = work.tile([128, 512], F32, tag="gftmp",
                                        bufs=2)
                        nc.vector.tensor_add(
                            tmp[:], pss[hf][:],
                            gb[:, hf * 512:(hf + 1) * 512])
                        nc.scalar.activation(
                            gt[:, hf * 512:(hf + 1) * 512], tmp[:],
                            ACTF.Sigmoid)

                # ---- GFCU ----
                nc.vector.tensor_add(s6v[:], s6v[:], bout_bc[:])
                ssum2 = work.tile([TPC, 1], F32, tag="ssum", bufs=2)
                sq2 = work.tile([TPC, D_], F32, tag="scratch")
                nc.scalar.activation(sq2[:], s6v[:], ACTF.Square,
                                     accum_out=ssum2[:])
                rr2 = work.tile([TPC, 1], F32, tag="rr", bufs=2)
                nc.scalar.activation(rr2[:], ssum2[:], ACTF.Sqrt,
                                     scale=1.0 / D_, bias=eps_sb[:])
                rinv2 = work.tile([TPC, 1], F32, tag="rinv", bufs=2)
                nc.vector.reciprocal(rinv2[:], rr2[:])
                nc.vector.scalar_tensor_tensor(
                    s6v[:], s6v[:], rinv2[:], s6g_bc[:], op0=ALU.mult,
                    op1=ALU.mult)
                mix = per.tile([TPC, D_], F32, tag="mix")
                nc.vector.tensor_mul(mix[:], g1[:], s6v[:])
                gg = work.tile([TPC, D_], F32, tag="scratch")
                nc.vector.tensor_mul(gg[:], g2[:], glcm_tm[:])
                nc.vector.tensor_add(mix[:], mix[:], gg[:])
                ssum3 = work.tile([TPC, 1], F32, tag="ssum", bufs=2)
                sq3 = work.tile([TPC, D_], F32, tag="scratch")
                nc.scalar.activation(sq3[:], mix[:], ACTF.Square,
                                     accum_out=ssum3[:])
                rr3 = work.tile([TPC, 1], F32, tag="rr", bufs=2)
                nc.scalar.activation(rr3[:], ssum3[:], ACTF.Sqrt,
                                     scale=1.0 / D_, bias=eps_sb[:])
                rinv3 = work.tile([TPC, 1], F32, tag="rinv", bufs=2)
                nc.vector.reciprocal(rinv3[:], rr3[:])
                h_sb = per.tile([TPC, D_], F32, tag="h_sb")
                nc.vector.scalar_tensor_tensor(
                    h_sb[:], mix[:], rinv3[:], gfg_bc[:], op0=ALU.mult,
                    op1=ALU.mult)
                nc.vector.tensor_add(h_sb[:], h_sb[:], x_sb[:])
                nc.sync.dma_start(h_out.ap(), h_sb[:])

                # ---- h^T (f32 logits; bf16 AllGather) ----
                ps_l = psC.tile([128, E_], F32, tag="psmall")
                for dt_i in range(8):
                    pst = psB.tile([128, 128], F32, tag="ptp", bufs=1)
                    nc.tensor.transpose(
                        pst[:], h_sb[:, dt_i * 128:(dt_i + 1) * 128],
                        ident[:])
                    ht32 = work.tile([128, TPC], F32, tag="ht32", bufs=2,
                                     name=f"ht32_{dt_i}")
                    nc.vector.tensor_copy(out=ht32[:], in_=pst[:])
                    nc.tensor.matmul(ps_l[:], ht32[:], wg_sb[dt_i][:],
                                     start=(dt_i == 0), stop=(dt_i == 7))
                    thb = per.tile([128, TPC], BF16, tag=f"ht{dt_i}",
                                   name=f"ht{dt_i}")
                    nc.scalar.copy(thb[:], ht32[:])
                    nc.sync.dma_start(
                        agin_h[dt_i * 128:(dt_i + 1) * 128, :], thb[:])
                nc.gpsimd.collective_compute(
                    "AllGather", ALU.bypass, replica_groups=rg,
                    ins=[agin_h[:].opt()], outs=[ag_h[:].opt()])

                # ---- top-2 routing ----
                logit = work.tile([TPC, E_], F32, tag="logit")
                nc.vector.tensor_copy(out=logit[:], in_=ps_l[:])
                max1 = work.tile([TPC, 1], F32, tag="max1")
                nc.vector.tensor_reduce(max1[:], logit[:], axis=AX.X,
                                        op=ALU.max)
                ge1 = work.tile([TPC, E_], F32, tag="ge1")
                nc.vector.tensor_scalar(ge1[:], logit[:], max1[:], None,
                                        op0=ALU.is_ge)
                lm = work.tile([TPC, E_], F32, tag="lm")
                nc.vector.scalar_tensor_tensor(lm[:], ge1[:], -1e30,
                                               logit[:], op0=ALU.mult,
                                               op1=ALU.add)
                max2 = work.tile([TPC, 1], F32, tag="max2")
                nc.vector.tensor_reduce(max2[:], lm[:], axis=AX.X,
                                        op=ALU.max)
                negm1 = work.tile([TPC, 1], F32, tag="negm1")
                nc.vector.tensor_scalar_mul(negm1[:], max1[:], -1.0)
                expd = work.tile([TPC, E_], F32, tag="expd")
                nc.scalar.activation(expd[:], logit[:], ACTF.Exp,
                                     bias=negm1[:])
                r2 = work.tile([TPC, 1], F32, tag="r2")
                nc.scalar.activation(r2[:], max2[:], ACTF.Exp,
                                     bias=negm1[:])
                zz = work.tile([TPC, 1], F32, tag="zz")
                nc.vector.tensor_scalar_add(zz[:], r2[:], 1.0)
                zi = work.tile([TPC, 1], F32, tag="zi")
                nc.vector.reciprocal(zi[:], zz[:])
                ge2 = work.tile([TPC, E_], F32, tag="ge2")
                nc.vector.tensor_scalar(ge2[:], logit[:], max2[:], None,
                                        op0=ALU.is_ge)
                w_sb = work.tile([TPC, E_], F32, tag="w_sb")
                nc.vector.scalar_tensor_tensor(w_sb[:], expd[:], zi[:],
                                               ge2[:], op0=ALU.mult,
                                               op1=ALU.mult)
                nc.sync.dma_start(w_out.ap(), w_sb[:])
                ps_wt = psB.tile([128, 128], F32, tag="ptp", bufs=1)
                nc.tensor.transpose(ps_wt[:E_, :], w_sb[:], ident[:])
                wt_sb = work.tile([E_, TPC], F32, tag="wt_sb")
                nc.vector.tensor_copy(out=wt_sb[:], in_=ps_wt[:E_, :])
                nc.sync.dma_start(a2a_in[:], wt_sb[:])
                nc.gpsimd.collective_compute(
                    "AllToAll", ALU.bypass, replica_groups=rg,
                    ins=[a2a_in[:].opt()], outs=[a2a_out[:].opt()])

                # ---- load h^T full (reuses xtf slots) ----
                ag_h_v = ag_h[:].rearrange("(r q) j -> q r j", r=NC_)
                ht_full = []
                for k in range(8):
                    t = per.tile([128, TOK], BF16,
                                 tag=(f"xtf{k}" if repeat == 1
                                      else f"htf{k}"),
                                 name=f"htf{k}")
                    nc.sync.dma_start(t[:], ag_h_v[k * 128:(k + 1) * 128])
                    ht_full.append(t)

                # ---- MoE sparse, per expert ----
                a2a_wrap = a2a_out[:].rearrange(
                    "(q two) (m p) -> two p q m", two=2, p=16)
                for ei in range(2):
                    we_wrap = work.tile([16, 64], F32, tag="we_wrap",
                                        bufs=2)
                    for q_ in range(8):
                        nc.sync.dma_start(we_wrap[:, q_ * 8:(q_ + 1) * 8],
                                          a2a_wrap[ei][:, q_, :])
                    mask = work.tile([16, 64], F32, tag="mask", bufs=2)
                    nc.vector.tensor_scalar(mask[:], we_wrap[:], 0.0, None,
                                            op0=ALU.is_gt)
                    val = work.tile([16, 64], F32, tag="val", bufs=2)
                    nc.vector.tensor_mul(val[:], mask[:], iota_f1[:])
                    nc.vector.tensor_scalar_add(val[:], val[:], -1.0)
                    ids_f = work.tile([16, CAP // 16], F32, tag="ids_f",
                                      bufs=2)
                    nf = work.tile([1, 1], U32, tag="nf", bufs=2)
                    nc.gpsimd.sparse_gather(ids_f[:], val[:],
                                            num_found=nf[:])
                    nc.sync.dma_start(nf_out.ap()[ei:ei + 1, :], nf[:])
                    nc.vector.tensor_scalar_min(ids_f[:], ids_f[:], 1023.0)
                    nc.vector.tensor_scalar_max(ids_f[:], ids_f[:], 0.0)
                    nc.sync.dma_start(ids_out.ap()[ei], ids_f[:])
                    ids_u = work.tile([16, CAP // 16], U16, tag="ids_u",
                                      bufs=2)
                    nc.vector.tensor_copy(out=ids_u[:], in_=ids_f[:])
                    idsr = work.tile([128, CAP // 16], U16, tag="idsr",
                                     bufs=2)
                    for g8 in range(8):
                        nc.sync.dma_start(idsr[g8 * 16:(g8 + 1) * 16, :],
                                          ids_u[:])
                    hg = []
                    for k in range(8):
                        t = work.tile([128, CAP], BF16, tag=f"hg{k}",
                                      bufs=1, name=f"hg{ei}_{k}")
                        nc.gpsimd.indirect_copy(t[:], ht_full[k][:],
                                                idsr[:], True)
                        hg.append(t)
                    # e1 = silu((hg^T W1)/64 + b1), feature-major
                    e1_bf = []
                    for colb in range(8):
                        w1t = []
                        for k in range(8):
                            wt = wstr.tile([128, 512], BF16, tag="wqf8",
                                           bufs=(16 if repeat == 1 else 8),
                                           name=f"w1_{ei}_{colb}_{k}")
                            nc.sync.dma_start(wt[:], w1.ap()[ei, colb, k])
                            w1t.append(wt)
                        for c4 in range(4):
                            col = colb * 4 + c4
                            pse = psC.tile([128, CAP], F32, tag="psmall",
                                           name=f"pse{ei}_{col}")
                            for k in range(8):
                                nc.tensor.matmul(
                                    pse[:],
                                    w1t[k][:, c4 * 128:(c4 + 1) * 128],
                                    hg[k][:], start=(k == 0),
                                    stop=(k == 7))
                            t = work.tile([128, CAP], BF16,
                                          tag=f"e1_{col}",
                                          name=f"e1_{ei}_{col}")
                            nc.scalar.activation(
                                t[:], pse[:], ACTF.Silu,
                                bias=b1_sb[ei][:, col:col + 1])
                            e1_bf.append(t)
                    # e2 = e1^T @ W2 (x64; host rescales)
                    psg = [psA.tile([128, 512], F32, tag="pnb",
                                    name=f"psg{ei}_{i}")
                           for i in range(2)] \
                        + [psB.tile([128, 512], F32, tag="pmid",
                                    name=f"psg{ei}_{i2 + 2}", bufs=2)
                           for i2 in range(2)]
                    for k2 in range(32):
                        wt = wstr.tile([128, D_], BF16, tag="w2s",
                                       bufs=(6 if repeat == 1 else 3),
                                       name=f"w2_{ei}_{k2}")
                        nc.sync.dma_start(wt[:], w2.ap()[ei, k2])
                        for tt_ in range(2):
                            nrow = 128 if tt_ == 0 else CAP - 128
                            for hf in range(2):
                                nc.tensor.matmul(
                                    psg[tt_ * 2 + hf][:nrow],
                                    e1_bf[k2][:, tt_ * 128:
                                              tt_ * 128 + nrow],
                                    wt[:, hf * 512:(hf + 1) * 512],
                                    start=(k2 == 0), stop=(k2 == 31))
                    for tt_ in range(2):
                        nrow = 128 if tt_ == 0 else CAP - 128
                        for hf in range(2):
                            cp = work.tile([128, 512], F32, tag="pcp",
                                           bufs=1,
                                           name=f"e2cp{ei}_{tt_}_{hf}")
                            nc.vector.tensor_copy(
                                out=cp[:nrow],
                                in_=psg[tt_ * 2 + hf][:nrow])
                            nc.sync.dma_start(
                                e2_out.ap()[ei,
                                            tt_ * 128:tt_ * 128 + nrow,
                                            hf * 512:(hf + 1) * 512],
                                cp[:nrow])

            for _rep in range(repeat):
                emit_body()

    nc.compile()
    return nc


# ---------------------------------------------------------------------------
def prep_inputs(inputs):
    f = {k: np.asarray(v, dtype=np.float32) for k, v in inputs.items()}
    xf = f["x"].reshape(TOK, D_)
    xT = np.ascontiguousarray(xf.T)
    perm = np.r_[np.arange(0, D_, 2), np.arange(1, D_, 2)]

    win_f = f["s6_Win"]
    bin_f = f["s6_bin"]
    WbF = win_f @ f["s6_WB"]
    WcF = win_f @ f["s6_WC"]
    WdF = win_f @ f["s6_Wd"]
    bBe = bin_f @ f["s6_WB"] + f["s6_bB"]
    bCe = bin_f @ f["s6_WC"] + f["s6_bC"]
    bde = bin_f @ f["s6_Wd"] + f["s6_bd"]

    wp_perm = np.concatenate([f["glcm_Wp"][:, :D_][:, perm],
                              f["glcm_Wp"][:, D_:][:, perm]], axis=1)
    bp_perm = np.concatenate([f["glcm_bp"][:D_][perm],
                              f["glcm_bp"][D_:][perm]])
    wdw_p = f["glcm_Wdw"][:, perm]
    bdw_p = f["glcm_bdw"][perm]
    wpb = np.zeros((8, 8, 128, 256), np.float32)
    for i in range(4):
        for k in range(8):
            wpb[i, k, :, :128] = wp_perm[k * 128:(k + 1) * 128,
                                         i * 128:(i + 1) * 128]
            wpb[i, k, :, 128:] = wp_perm[k * 128:(k + 1) * 128,
                                         512 + i * 128:512 + (i + 1) * 128]
    for j in range(4):
        for k in range(8):
            wpb[4 + j, k] = wp_perm[k * 128:(k + 1) * 128,
                                    D_ + j * 256:D_ + (j + 1) * 256]

    sinT = np.ascontiguousarray(f["sin"].T)
    cosT = np.ascontiguousarray(f["cos"].T)
    sin_pad = np.zeros((D_ // 2, T_ + 2 * HALO), np.float32)
    cos_pad = np.zeros((D_ // 2, T_ + 2 * HALO), np.float32)
    sin_pad[:, HALO:HALO + T_] = sinT
    cos_pad[:, HALO:HALO + T_] = cosT

    shared = {
        "xt_full": xT.astype(bf16),
        "wp_blk": wpb.astype(bf16),
        "bp_h": np.ascontiguousarray(bp_perm.reshape(16, 128).T),
        "wdw_h": np.ascontiguousarray(
            wdw_p.T.reshape(8, 128, K7).transpose(1, 0, 2).reshape(128, -1)),
        "bdw_h": np.ascontiguousarray(bdw_p.reshape(8, 128).T),
        "glcm_g": f["glcm_g"],
        "gfw1": f["gf_W1"].astype(bf16),
        "gfw2": f["gf_W2"].astype(bf16),
        "gfb1": f["gf_b1"], "gfb2": f["gf_b2"], "gf_g": f["gf_g"],
        "bout": f["s6_bout"], "s6g": f["s6_g"],
        "wg": f["moe_Wg"],
        "wout": f["s6_Wout"].astype(bf16),
    }

    A_full = -np.exp(f["s6_Alog"])

    in_maps = []
    for c in range(NC_):
        m = dict(shared)
        t0 = (c % 4) * TPC
        b0 = c // 4
        tok0 = c * TPC
        sc = slice(c * SPC, (c + 1) * SPC)

        seg = xT[:, b0 * T_:(b0 + 1) * T_]
        pad = np.zeros((D_, T_ + 2 * HALO), np.float32)
        pad[:, HALO:HALO + T_] = seg
        m["xt_halo"] = pad[:, t0:t0 + THW].astype(bf16)
        m["x_tm"] = np.ascontiguousarray(xf[tok0:tok0 + TPC])
        m["sin_t"] = np.ascontiguousarray(sin_pad[:, t0:t0 + THW])
        m["cos_t"] = np.ascontiguousarray(cos_pad[:, t0:t0 + THW])
        m["win_u"] = np.ascontiguousarray(win_f[:, sc]).astype(bf16)
        m["bin_u"] = np.ascontiguousarray(bin_f[sc][:, None])
        m["wdf"] = np.ascontiguousarray(WdF[:, sc]).astype(bf16)
        m["bd_h"] = np.ascontiguousarray(bde[sc][:, None])
        wb_r = WbF.reshape(D_, S_, N_)[:, sc, :].transpose(0, 2, 1)
        m["wbf"] = np.ascontiguousarray(
            wb_r.reshape(D_, 4, 512).transpose(1, 0, 2)
            .reshape(4, 8, 128, 512)).astype(bf16)
        wc_r = WcF.reshape(D_, S_, N_)[:, sc, :].transpose(0, 2, 1)
        m["wcf"] = np.ascontiguousarray(
            wc_r.reshape(D_, 4, 512).transpose(1, 0, 2)
            .reshape(4, 8, 128, 512)).astype(bf16)
        m["bb_h"] = np.ascontiguousarray(bBe.reshape(S_, N_)[sc])
        m["bc_h"] = np.ascontiguousarray(bCe.reshape(S_, N_)[sc])
        m["a_mat"] = np.ascontiguousarray(A_full[sc])
        ee = [2 * c, 2 * c + 1]
        w1s = f["moe_W1"][ee].reshape(2, 8, 128, 8, 512).transpose(
            0, 3, 1, 2, 4)
        m["w1"] = np.ascontiguousarray(w1s).astype(bf16)
        m["b1_h"] = np.ascontiguousarray(
            f["moe_b1"][ee].reshape(2, 32, 128).transpose(0, 2, 1))
        m["w2"] = np.ascontiguousarray(
            f["moe_W2"][ee].reshape(2, 32, 128, D_)).astype(bf16)
        in_maps.append(m)
    return in_maps


def postprocess(inputs, results):
    b2 = np.asarray(inputs["moe_b2"], dtype=np.float32)
    h_full = np.concatenate([r["h_out"] for r in results], axis=0)
    w_full = np.concatenate([r["w_out"] for r in results], axis=0)
    out = h_full.copy()
    for c in range(NC_):
        r = results[c]
        for ei in range(2):
            e = 2 * c + ei
            n = int(r["nf_out"][ei, 0])
            if n == 0:
                continue
            nd = min(n, CAP)
            ids_flat = r["ids_out"][ei].T.reshape(-1)[:nd].astype(np.int64)
            vals = r["e2_out"][ei, :nd] + b2[e][None, :]
            out[ids_flat] += w_full[ids_flat, e][:, None] * vals
            if n > CAP:
                # capacity overflow (~never): host computes the tail exactly
                sel = np.where(w_full[:, e] > 0)[0]
                missed = np.setdiff1d(sel, ids_flat)
                if len(missed):
                    W1 = np.asarray(inputs["moe_W1"][e], np.float32)
                    W2 = np.asarray(inputs["moe_W2"][e], np.float32)
                    b1v = np.asarray(inputs["moe_b1"][e], np.float32)
                    hh = h_full[missed]
                    a1 = hh @ W1 + b1v
                    a1 = a1 / (1.0 + np.exp(-a1))
                    ee2 = a1 @ W2 + b2[e]
                    out[missed] += w_full[missed, e][:, None] * ee2
    return out.reshape(B_, T_, D_)


_CACHE = {}


def kernel(**inputs):
    if "nc" not in _CACHE:
        _CACHE["nc"] = build_program()
    nc = _CACHE["nc"]
    in_maps = prep_inputs(inputs)
    res = bass_utils.run_bass_kernel_spmd(nc, in_maps,
                                          core_ids=list(range(NC_)))
    return postprocess(inputs, res.results)


if __name__ == "__main__":
    print("building...")
    build_program()
    print("built ok")


# revision 36
# speedup vs baseline: 13.4723x; 13.4723x over previous
"""Trainium2 Bass kernel for nn_AetheriusCoreBlock (8-core SPMD).

Design:
  - Host fuses xi = x@Win+bin into the downstream S6 weights (WbF = Win@WB,
    WcF = Win@WC, WdF = Win@Wd; exact algebra), so the device works straight
    from x^T (full, bf16, SBUF-resident).
  - Channel-sharded S6 (128 of 1024 state channels/core): fused WB/WC
    projections (n-major columns, quarter-blocked single-read streaming),
    selective scan via tensor_tensor_scan (HW prefix scan), then an AllToAll
    routes y back to token shards and y@Wout runs token-locally.
  - Token-sharded front (128 tokens/core): GLCM branch (rotary channels
    de-interleaved via host-permuted Wp, depthwise conv as 7 fused
    shifted-window ops, PE-transpose + strided write un-permutes), GFCU gates.
  - MoE: expert-sharded (2 experts/core). Routing logits in f32 (top-2
    selection must match the f32 reference), routing weights exchanged with
    AllToAll; per expert the selected tokens are compacted with sparse_gather
    and gathered with indirect_copy (capacity CAP=192, host fallback on
    overflow); expert weights in fp8(e4m3, x64 scale) to halve DMA; outputs
    returned unweighted and combined on host.

kernel(**inputs) takes FULL inputs (as from setup_inputs) and returns the
FULL [2, 512, 1024] float32 output.
"""

import sys
import numpy as np

sys.path.insert(0, "/opt/trn_rl_repo")

import ml_dtypes

bf16 = ml_dtypes.bfloat16
fp8np = ml_dtypes.float8_e4m3

from concourse import bass, bacc, mybir, tile  # noqa: E402
from concourse import bass_utils  # noqa: E402
from concourse.masks import make_identity  # noqa: E402

F32 = mybir.dt.float32
BF16 = mybir.dt.bfloat16
FP8 = mybir.dt.float8e4
U16 = mybir.dt.uint16
U32 = mybir.dt.uint32
I32 = mybir.dt.int32
ALU = mybir.AluOpType
ACTF = mybir.ActivationFunctionType
AX = mybir.AxisListType

NC_ = 8
B_, T_, D_ = 2, 512, 1024
S_, N_, E_ = 1024, 16, 16
K7 = 7
TOK = B_ * T_
TPC = TOK // NC_     # 128 tokens/core
SPC = S_ // NC_      # 128 state channels/core
CAP = 192            # token capacity per expert (mean 128, +6 sigma)
HALO = 3
THW = TPC + 2 * HALO  # 134
EPS = 1e-8
MOE_WSCALE = 64.0

DEBUG = False
REPEAT = 1           # timing only: emit the body N times in one program


def bcast_ap(t, n_part=128):
    ap = t.ap()[None, :]
    ap.ap[0] = [0, n_part]
    return ap


def build_program(repeat=None):
    repeat = REPEAT if repeat is None else repeat
    nc = bacc.Bacc("TRN2", target_bir_lowering=False, debug=False,
                   num_devices=NC_)

    def inp(name, shape, dt=F32):
        return nc.dram_tensor(name, list(shape), dt, kind="ExternalInput")

    def outp(name, shape, dt=F32):
        return nc.dram_tensor(name, list(shape), dt, kind="ExternalOutput")

    xt_halo = inp("xt_halo", [D_, THW], BF16)
    xt_full = inp("xt_full", [D_, TOK], BF16)
    x_tm = inp("x_tm", [TPC, D_])
    sin_t = inp("sin_t", [D_ // 2, THW])
    cos_t = inp("cos_t", [D_ // 2, THW])
    win_u = inp("win_u", [D_, SPC], BF16)
    bin_u = inp("bin_u", [128, 1])
    wp_blk = inp("wp_blk", [8, 8, 128, 256], BF16)
    bp_h = inp("bp_h", [128, 16])
    wdw_h = inp("wdw_h", [128, 8 * K7])
    bdw_h = inp("bdw_h", [128, 8])
    glcm_g = inp("glcm_g", [D_])
    gfw1 = inp("gfw1", [D_, D_], BF16)
    gfw2 = inp("gfw2", [D_, D_], BF16)
    gfb1 = inp("gfb1", [D_])
    gfb2 = inp("gfb2", [D_])
    gf_g = inp("gf_g", [D_])
    wdf = inp("wdf", [D_, SPC], BF16)
    bd_h = inp("bd_h", [128, 1])
    wbf = inp("wbf", [4, 8, 128, 512], BF16)
    wcf = inp("wcf", [4, 8, 128, 512], BF16)
    bb_h = inp("bb_h", [128, 16])
    bc_h = inp("bc_h", [128, 16])
    a_mat = inp("a_mat", [128, 16])
    wout = inp("wout", [S_, D_], BF16)
    bout = inp("bout", [D_])
    s6g = inp("s6g", [D_])
    wg = inp("wg", [D_, E_])
    w1 = inp("w1", [2, 8, 8, 128, 512], BF16)
    b1_h = inp("b1_h", [2, 128, 32])
    w2 = inp("w2", [2, 32, 128, D_], BF16)

    h_out = outp("h_out", [TPC, D_])
    w_out = outp("w_out", [TPC, E_])
    ids_out = outp("ids_out", [2, 16, CAP // 16])
    nf_out = outp("nf_out", [2, 1], U32)
    e2_out = outp("e2_out", [2, CAP, D_])

    rg = [list(range(NC_))]

    with tile.TileContext(nc) as tc:
        with (
            tc.tile_pool(name="consts", bufs=1) as consts,
            tc.tile_pool(name="per", bufs=1) as per,
            tc.tile_pool(name="wstr", bufs=1) as wstr,
            tc.tile_pool(name="work", bufs=1) as work,
            tc.tile_pool(name="psA", bufs=3, space="PSUM") as psA,
            tc.tile_pool(name="psB", bufs=2, space="PSUM") as psB,
            tc.tile_pool(name="psC", bufs=2, space="PSUM") as psC,
            tc.tile_pool(name="dram", bufs=1, space="DRAM") as dram,
        ):
            ident = consts.tile([128, 128], F32, tag="ident")
            make_identity(nc, ident[:])
            eps_sb = consts.tile([128, 1], F32, tag="eps_sb")
            nc.vector.memset(eps_sb[:], EPS)

            def cload(name, src_ap, shape, dt=F32):
                t = consts.tile(list(shape), dt, tag=name, name=name)
                eng = nc.gpsimd if (dt == BF16 and src_ap.dtype == F32) \
                    else nc.sync
                eng.dma_start(t[:], src_ap)
                return t

            binu_sb = cload("binu_sb", bin_u.ap(), [128, 1])
            bp_sb = cload("bp_sb", bp_h.ap(), [128, 16])
            bd_sb = cload("bd_sb", bd_h.ap(), [128, 1])
            bb_sb = cload("bb_sb", bb_h.ap(), [128, 16])
            bc_sb = cload("bc_sb", bc_h.ap(), [128, 16])
            a_sb = cload("a_sb", a_mat.ap(), [128, 16])
            wdw_sb = cload("wdw_sb", wdw_h.ap(), [128, 8 * K7])
            bdw_sb = cload("bdw_sb", bdw_h.ap(), [128, 8])
            b1_sb = [cload(f"b1_sb{e}", b1_h.ap()[e], [128, 32])
                     for e in (0, 1)]
            glcmg_bc = cload("glcmg_bc", bcast_ap(glcm_g), [128, D_], BF16)
            s6g_bc = cload("s6g_bc", bcast_ap(s6g), [128, D_], BF16)
            gfg_bc = cload("gfg_bc", bcast_ap(gf_g), [128, D_], BF16)
            bout_bc = cload("bout_bc", bcast_ap(bout), [128, D_], BF16)
            gfb1_bc = cload("gfb1_bc", bcast_ap(gfb1), [128, D_], BF16)
            gfb2_bc = cload("gfb2_bc", bcast_ap(gfb2), [128, D_], BF16)
            sin_sb = [cload(f"sin{i}", sin_t.ap()[i * 128:(i + 1) * 128, :],
                            [128, THW]) for i in range(4)]
            cos_sb = [cload(f"cos{i}", cos_t.ap()[i * 128:(i + 1) * 128, :],
                            [128, THW]) for i in range(4)]
            wg_sb = [cload(f"wg_sb{k}", wg.ap()[k * 128:(k + 1) * 128, :],
                           [128, E_], F32) for k in range(8)]

            xt = []
            for k in range(8):
                t = per.tile([128, THW], BF16, tag=f"xt{k}")
                nc.sync.dma_start(t[:], xt_halo.ap()[k * 128:(k + 1) * 128, :])
                xt.append(t)
            xt_c = [t[:, HALO:HALO + TPC] for t in xt]

            xtf = []
            for k in range(8):
                t = per.tile([128, TOK], BF16, tag=f"xtf{k}")
                nc.sync.dma_start(t[:], xt_full.ap()[k * 128:(k + 1) * 128, :])
                xtf.append(t)

            x_sb = per.tile([TPC, D_], F32, tag="x_sb")
            nc.sync.dma_start(x_sb[:], x_tm.ap())


            iota_i = per.tile([16, 64], I32, tag="iota_i")
            nc.gpsimd.iota(iota_i[:], pattern=[[16, 64]], base=0,
                           channel_multiplier=1)
            iota_f1 = per.tile([16, 64], F32, tag="iota_f1")
            nc.vector.tensor_copy(out=iota_f1[:], in_=iota_i[:])
            nc.vector.tensor_scalar_add(iota_f1[:], iota_f1[:], 1.0)

            def emit_body():
                ya_in = dram.tile([NC_ * 128, TPC], BF16, name="ya_in")
                ya_out = dram.tile([NC_ * 128, TPC], BF16, name="ya_out")
                agin_h = dram.tile([D_, TPC], BF16, name="agin_h")
                ag_h = dram.tile([NC_ * D_, TPC], BF16,
                                 addr_space="Shared", name="ag_h")
                a2a_in = dram.tile([2 * NC_, TPC], F32, name="a2a_in")
                a2a_out = dram.tile([2 * NC_, TPC], F32, name="a2a_out")
                # ---- delta / u (all tokens, f32) ----
                delta = per.tile([128, TOK], F32, tag="delta")
                u_sb = per.tile([128, TOK], F32, tag="u_sb")
                for wi, (wsrc, dst, bias, is_sp) in enumerate((
                        (wdf, delta, bd_sb, True),
                        (win_u, u_sb, binu_sb, False))):
                    wt_l = []
                    for k in range(8):
                        t = wstr.tile([128, SPC], BF16, tag="lhs128", bufs=16,
                                      name=f"du_w{wi}_{k}")
                        nc.sync.dma_start(t[:],
                                          wsrc.ap()[k * 128:(k + 1) * 128, :])
                        wt_l.append(t)
                    for hf in range(2):
                        ps = psB.tile([128, 512], F32, tag="pmid")
                        for k in range(8):
                            nc.tensor.matmul(
                                ps[:], wt_l[k][:],
                                xtf[k][:, hf * 512:(hf + 1) * 512],
                                start=(k == 0), stop=(k == 7))
                        if is_sp:
                            spt = work.tile([128, 512], F32, tag="gftmp",
                                            bufs=2, name=f"spt{hf}")
                            nc.scalar.activation(spt[:], ps[:], ACTF.Exp,
                                                 bias=bias[:])
                            nc.scalar.activation(
                                dst[:, hf * 512:(hf + 1) * 512], spt[:],
                                ACTF.Ln, bias=1.0)
                        else:
                            nc.scalar.activation(
                                dst[:, hf * 512:(hf + 1) * 512], ps[:],
                                ACTF.Identity, bias=bias[:])
                du = per.tile([128, TOK], F32, tag="du")
                nc.vector.tensor_mul(du[:], delta[:], u_sb[:])

                # ---- S6: fused WB/WC + scan, per (nq, n4, b) ----
                y_sb = per.tile([128, TOK], F32, tag="y_sb")
                for nq in range(4):
                    wbq, wcq = [], []
                    for k in range(8):
                        t = wstr.tile([128, 512], BF16, tag="wq", bufs=16,
                                      name=f"wbq{nq}_{k}")
                        nc.sync.dma_start(t[:], wbf.ap()[nq, k])
                        wbq.append(t)
                        t2 = wstr.tile([128, 512], BF16, tag="wq", bufs=16,
                                       name=f"wcq{nq}_{k}")
                        nc.sync.dma_start(t2[:], wcf.ap()[nq, k])
                        wcq.append(t2)
                    for n4 in range(4):
                        n = nq * 4 + n4
                        for b in range(B_):
                            sl = slice(b * T_, (b + 1) * T_)
                            ps_b = psA.tile([128, T_], F32, tag="pnb")
                            for k in range(8):
                                nc.tensor.matmul(
                                    ps_b[:],
                                    wbq[k][:, n4 * 128:(n4 + 1) * 128],
                                    xtf[k][:, sl], start=(k == 0),
                                    stop=(k == 7))
                            a_t = work.tile([128, T_], F32, tag="a_t",
                                            bufs=2)
                            nc.scalar.activation(a_t[:], delta[:, sl],
                                                 ACTF.Exp,
                                                 scale=a_sb[:, n:n + 1])
                            dbu = work.tile([128, T_], F32, tag="dbu",
                                            bufs=2)
                            nc.vector.scalar_tensor_tensor(
                                dbu[:], ps_b[:], bb_sb[:, n:n + 1],
                                du[:, sl], op0=ALU.add, op1=ALU.mult)
                            H_t = work.tile([128, T_], F32, tag="H_t",
                                            bufs=2)
                            nc.vector.tensor_tensor_scan(
                                H_t[:], a_t[:], dbu[:], initial=0.0,
                                op0=ALU.mult, op1=ALU.add)
                            ps_c = psA.tile([128, T_], F32, tag="pnb")
                            for k in range(8):
                                nc.tensor.matmul(
                                    ps_c[:],
                                    wcq[k][:, n4 * 128:(n4 + 1) * 128],
                                    xtf[k][:, sl], start=(k == 0),
                                    stop=(k == 7))
                            if n == 0:
                                nc.vector.scalar_tensor_tensor(
                                    y_sb[:, sl], ps_c[:],
                                    bc_sb[:, n:n + 1], H_t[:],
                                    op0=ALU.add, op1=ALU.mult)
                            else:
                                yt = work.tile([128, T_], F32, tag="yt",
                                               bufs=2)
                                nc.vector.scalar_tensor_tensor(
                                    yt[:], ps_c[:], bc_sb[:, n:n + 1],
                                    H_t[:], op0=ALU.add, op1=ALU.mult)
                                nc.vector.tensor_add(y_sb[:, sl], yt[:],
                                                     y_sb[:, sl])

                # ---- y AllToAll (token-shard routing) + local Wout ----
                y_bf = per.tile([128, TOK], BF16, tag="y_bf")
                nc.scalar.copy(y_bf[:], y_sb[:])
                for j in range(8):
                    nc.sync.dma_start(
                        ya_in[:].rearrange("(c p) t -> c p t", c=8)[j],
                        y_bf[:, j * 128:(j + 1) * 128])
                nc.gpsimd.collective_compute(
                    "AllToAll", ALU.bypass, replica_groups=rg,
                    ins=[ya_in[:].opt()], outs=[ya_out[:].opt()])
                ya_v = ya_out[:].rearrange("(r p) t -> r p t", r=8)
                yfull = []
                for k in range(8):
                    t = per.tile([128, TPC], BF16, tag=f"yfull{k}",
                                 name=f"yfull{k}")
                    nc.sync.dma_start(t[:], ya_v[k])
                    yfull.append(t)
                s6v = per.tile([TPC, D_], F32, tag="s6v")
                pso = [psB.tile([128, 512], F32, tag="pmid",
                                name=f"pso{hf}") for hf in range(2)]
                for k in range(8):
                    wt = wstr.tile([128, D_], BF16, tag="wmed", bufs=3,
                                   name=f"wout_{k}")
                    nc.sync.dma_start(wt[:],
                                      wout.ap()[k * 128:(k + 1) * 128, :])
                    for hf in range(2):
                        nc.tensor.matmul(pso[hf][:], yfull[k][:],
                                         wt[:, hf * 512:(hf + 1) * 512],
                                         start=(k == 0), stop=(k == 7))
                for hf in range(2):
                    nc.vector.tensor_copy(
                        out=s6v[:, hf * 512:(hf + 1) * 512], in_=pso[hf][:])

                # ---- GLCM ----
                wp_cache = {}

                def p_psum(blk, half):
                    ps = psC.tile([128, THW], F32, tag="psmall")
                    for k in range(8):
                        if half == 0:
                            wt = wstr.tile([128, 256], BF16, tag="wp_s",
                                           bufs=16, name=f"wp{blk}_{k}")
                            nc.sync.dma_start(wt[:], wp_blk.ap()[blk, k])
                            wp_cache[(blk, k)] = wt
                        wt = wp_cache[(blk, k)]
                        nc.tensor.matmul(
                            ps[:], wt[:, half * 128:(half + 1) * 128],
                            xt[k][:], start=(k == 0), stop=(k == 7))
                    return ps

                xr = [per.tile([128, THW], F32, tag=f"xr{i}", name=f"xr{i}")
                      for i in range(8)]
                for i in range(4):
                    ps_e = p_psum(i, 0)
                    ps_o = p_psum(i, 1)
                    t1 = work.tile([128, THW], F32, tag="rot1", bufs=2)
                    t2 = work.tile([128, THW], F32, tag="rot2", bufs=2)
                    nc.vector.scalar_tensor_tensor(
                        t1[:], ps_e[:], bp_sb[:, i:i + 1], cos_sb[i][:],
                        op0=ALU.add, op1=ALU.mult)
                    nc.vector.scalar_tensor_tensor(
                        t2[:], ps_o[:], bp_sb[:, i + 4:i + 5], sin_sb[i][:],
                        op0=ALU.add, op1=ALU.mult)
                    nc.vector.tensor_sub(xr[i][:], t1[:], t2[:])
                    nc.vector.scalar_tensor_tensor(
                        t1[:], ps_e[:], bp_sb[:, i:i + 1], sin_sb[i][:],
                        op0=ALU.add, op1=ALU.mult)
                    nc.vector.scalar_tensor_tensor(
                        t2[:], ps_o[:], bp_sb[:, i + 4:i + 5], cos_sb[i][:],
                        op0=ALU.add, op1=ALU.mult)
                    nc.vector.tensor_add(xr[i + 4][:], t1[:], t2[:])

                glcm_tm = per.tile([TPC, D_], F32, tag="glcm_tm")
                for dt_i in range(8):
                    acc = work.tile([128, TPC], F32, tag="convacc", bufs=2)
                    nc.vector.tensor_scalar_mul(
                        acc[:], xr[dt_i][:, 0:TPC],
                        wdw_sb[:, dt_i * K7:dt_i * K7 + 1])
                    for k in range(1, K7):
                        nc.vector.scalar_tensor_tensor(
                            acc[:], xr[dt_i][:, k:k + TPC],
                            wdw_sb[:, dt_i * K7 + k:dt_i * K7 + k + 1],
                            acc[:], op0=ALU.mult, op1=ALU.add)
                    sil = work.tile([128, TPC], F32, tag="convsil", bufs=2)
                    nc.scalar.activation(sil[:], acc[:], ACTF.Silu,
                                         bias=bdw_sb[:, dt_i:dt_i + 1])
                    ps_xb = p_psum(4 + dt_i // 2, dt_i % 2)
                    sg = work.tile([128, TPC], F32, tag="convsg", bufs=2)
                    nc.scalar.activation(sg[:], ps_xb[:, HALO:HALO + TPC],
                                         ACTF.Sigmoid,
                                         bias=bp_sb[:, 8 + dt_i:9 + dt_i])
                    gated = work.tile([128, TPC], F32, tag="gated", bufs=2)
                    nc.vector.tensor_mul(gated[:], sil[:], sg[:])
                    pst = psB.tile([128, 128], F32, tag="ptp", bufs=1)
                    nc.tensor.transpose(pst[:], gated[:], ident[:])
                    half = dt_i % 4
                    even = dt_i < 4
                    view = glcm_tm[:].rearrange("p (c two) -> p c two",
                                                two=2)
                    dst = view[:, half * 128:(half + 1) * 128,
                               0 if even else 1]
                    nc.vector.tensor_copy(out=dst, in_=pst[:])
                sq = work.tile([TPC, D_], F32, tag="scratch")
                ssum = work.tile([TPC, 1], F32, tag="ssum", bufs=2)
                nc.scalar.activation(sq[:], glcm_tm[:], ACTF.Square,
                                     accum_out=ssum[:])
                rr = work.tile([TPC, 1], F32, tag="rr", bufs=2)
                nc.scalar.activation(rr[:], ssum[:], ACTF.Sqrt,
                                     scale=1.0 / D_, bias=eps_sb[:])
                rinv = work.tile([TPC, 1], F32, tag="rinv", bufs=2)
                nc.vector.reciprocal(rinv[:], rr[:])
                nc.vector.scalar_tensor_tensor(
                    glcm_tm[:], glcm_tm[:], rinv[:], glcmg_bc[:],
                    op0=ALU.mult, op1=ALU.mult)

                # ---- gf gates ----
                g1 = per.tile([TPC, D_], F32, tag="g1")
                g2 = per.tile([TPC, D_], F32, tag="g2")
                for gi, (gt, wgf, gb) in enumerate(
                        ((g1, gfw1, gfb1_bc), (g2, gfw2, gfb2_bc))):
                    pss = [psB.tile([128, 512], F32, tag="pmid",
                                    name=f"gps{gi}_{hf}")
                           for hf in range(2)]
                    for k in range(8):
                        wt = wstr.tile([128, D_], BF16, tag="wmed", bufs=3,
                                       name=f"gf{gi}_{k}")
                        nc.sync.dma_start(
                            wt[:], wgf.ap()[k * 128:(k + 1) * 128, :])
                        for hf in range(2):
                            nc.tensor.matmul(
                                pss[hf][:], xt_c[k],
                                wt[:, hf * 512:(hf + 1) * 512],
                                start=(k == 0), stop=(k == 7))
                    for hf in range(2):
                        tmp = work.tile([128, 512], F32, tag="gftmp",
                                        bufs=2)
                        nc.vector.tensor_add(
                            tmp[:], pss[hf][:],
                            gb[:, hf * 512:(hf + 1) * 512])
                        nc.scalar.activation(
                            gt[:, hf * 512:(hf + 1) * 512], tmp[:],
                            ACTF.Sigmoid)

                # ---- GFCU ----
                nc.vector.tensor_add(s6v[:], s6v[:], bout_bc[:])
                ssum2 = work.tile([TPC, 1], F32, tag="ssum", bufs=2)
                sq2 = work.tile([TPC, D_], F32, tag="scratch")
                nc.scalar.activation(sq2[:], s6v[:], ACTF.Square,
                                     accum_out=ssum2[:])
                rr2 = work.tile([TPC, 1], F32, tag="rr", bufs=2)
                nc.scalar.activation(rr2[:], ssum2[:], ACTF.Sqrt,
                                     scale=1.0 / D_, bias=eps_sb[:])
                rinv2 = work.tile([TPC, 1], F32, tag="rinv", bufs=2)
                nc.vector.reciprocal(rinv2[:], rr2[:])
                nc.vector.scalar_tensor_tensor(
                    s6v[:], s6v[:], rinv2[:], s6g_bc[:], op0=ALU.mult,
                    op1=ALU.mult)
                mix = per.tile([TPC, D_], F32, tag="mix")
                nc.vector.tensor_mul(mix[:], g1[:], s6v[:])
                gg = work.tile([TPC, D_], F32, tag="scratch")
                nc.vector.tensor_mul(gg[:], g2[:], glcm_tm[:])
                nc.vector.tensor_add(mix[:], mix[:], gg[:])
                ssum3 = work.tile([TPC, 1], F32, tag="ssum", bufs=2)
                sq3 = work.tile([TPC, D_], F32, tag="scratch")
                nc.scalar.activation(sq3[:], mix[:], ACTF.Square,
                                     accum_out=ssum3[:])
                rr3 = work.tile([TPC, 1], F32, tag="rr", bufs=2)
                nc.scalar.activation(rr3[:], ssum3[:], ACTF.Sqrt,
                                     scale=1.0 / D_, bias=eps_sb[:])
                rinv3 = work.tile([TPC, 1], F32, tag="rinv", bufs=2)
                nc.vector.reciprocal(rinv3[:], rr3[:])
                h_sb = per.tile([TPC, D_], F32, tag="h_sb")
                nc.vector.scalar_tensor_tensor(
                    h_sb[:], mix[:], rinv3[:], gfg_bc[:], op0=ALU.mult,
                    op1=ALU.mult)
                nc.vector.tensor_add(h_sb[:], h_sb[:], x_sb[:])
                nc.sync.dma_start(h_out.ap(), h_sb[:])

                # ---- h^T (f32 logits; bf16 AllGather) ----
                ps_l = psC.tile([128, E_], F32, tag="psmall")
                for dt_i in range(8):
                    pst = psB.tile([128, 128], F32, tag="ptp", bufs=1)
                    nc.tensor.transpose(
                        pst[:], h_sb[:, dt_i * 128:(dt_i + 1) * 128],
                        ident[:])
                    ht32 = work.tile([128, TPC], F32, tag="ht32", bufs=2,
                                     name=f"ht32_{dt_i}")
                    nc.vector.tensor_copy(out=ht32[:], in_=pst[:])
                    nc.tensor.matmul(ps_l[:], ht32[:], wg_sb[dt_i][:],
                                     start=(dt_i == 0), stop=(dt_i == 7))
                    thb = per.tile([128, TPC], BF16, tag=f"ht{dt_i}",
                                   name=f"ht{dt_i}")
                    nc.scalar.copy(thb[:], ht32[:])
                    nc.sync.dma_start(
                        agin_h[dt_i * 128:(dt_i + 1) * 128, :], thb[:])
                nc.gpsimd.collective_compute(
                    "AllGather", ALU.bypass, replica_groups=rg,
                    ins=[agin_h[:].opt()], outs=[ag_h[:].opt()])

                # ---- load h^T full (reuses xtf slots) ----
                ag_h_v = ag_h[:].rearrange("(r q) j -> q r j", r=NC_)
                ht_full = []
                for k in range(8):
                    t = per.tile([128, TOK], BF16,
                                 tag=(f"xtf{k}" if repeat == 1
                                      else f"htf{k}"),
                                 name=f"htf{k}")
                    nc.sync.dma_start(t[:], ag_h_v[k * 128:(k + 1) * 128])
                    ht_full.append(t)

                # ---- top-2 routing ----
                logit = work.tile([TPC, E_], F32, tag="logit")
                nc.vector.tensor_copy(out=logit[:], in_=ps_l[:])
                max1 = work.tile([TPC, 1], F32, tag="max1")
                nc.vector.tensor_reduce(max1[:], logit[:], axis=AX.X,
                                        op=ALU.max)
                ge1 = work.tile([TPC, E_], F32, tag="ge1")
                nc.vector.tensor_scalar(ge1[:], logit[:], max1[:], None,
                                        op0=ALU.is_ge)
                lm = work.tile([TPC, E_], F32, tag="lm")
                nc.vector.scalar_tensor_tensor(lm[:], ge1[:], -1e30,
                                               logit[:], op0=ALU.mult,
                                               op1=ALU.add)
                max2 = work.tile([TPC, 1], F32, tag="max2")
                nc.vector.tensor_reduce(max2[:], lm[:], axis=AX.X,
                                        op=ALU.max)
                negm1 = work.tile([TPC, 1], F32, tag="negm1")
                nc.vector.tensor_scalar_mul(negm1[:], max1[:], -1.0)
                expd = work.tile([TPC, E_], F32, tag="expd")
                nc.scalar.activation(expd[:], logit[:], ACTF.Exp,
                                     bias=negm1[:])
                r2 = work.tile([TPC, 1], F32, tag="r2")
                nc.scalar.activation(r2[:], max2[:], ACTF.Exp,
                                     bias=negm1[:])
                zz = work.tile([TPC, 1], F32, tag="zz")
                nc.vector.tensor_scalar_add(zz[:], r2[:], 1.0)
                zi = work.tile([TPC, 1], F32, tag="zi")
                nc.vector.reciprocal(zi[:], zz[:])
                ge2 = work.tile([TPC, E_], F32, tag="ge2")
                nc.vector.tensor_scalar(ge2[:], logit[:], max2[:], None,
                                        op0=ALU.is_ge)
                w_sb = work.tile([TPC, E_], F32, tag="w_sb")
                nc.vector.scalar_tensor_tensor(w_sb[:], expd[:], zi[:],
                                               ge2[:], op0=ALU.mult,
                                               op1=ALU.mult)
                nc.sync.dma_start(w_out.ap(), w_sb[:])
                ps_wt = psB.tile([128, 128], F32, tag="ptp", bufs=1)
                nc.tensor.transpose(ps_wt[:E_, :], w_sb[:], ident[:])
                wt_sb = work.tile([E_, TPC], F32, tag="wt_sb")
                nc.vector.tensor_copy(out=wt_sb[:], in_=ps_wt[:E_, :])
                nc.sync.dma_start(a2a_in[:], wt_sb[:])
                nc.gpsimd.collective_compute(
                    "AllToAll", ALU.bypass, replica_groups=rg,
                    ins=[a2a_in[:].opt()], outs=[a2a_out[:].opt()])

                # ---- MoE sparse, per expert ----
                a2a_wrap = a2a_out[:].rearrange(
                    "(q two) (m p) -> two p q m", two=2, p=16)
                for ei in range(2):
                    we_wrap = work.tile([16, 64], F32, tag="we_wrap",
                                        bufs=2)
                    for q_ in range(8):
                        nc.sync.dma_start(we_wrap[:, q_ * 8:(q_ + 1) * 8],
                                          a2a_wrap[ei][:, q_, :])
                    mask = work.tile([16, 64], F32, tag="mask", bufs=2)
                    nc.vector.tensor_scalar(mask[:], we_wrap[:], 0.0, None,
                                            op0=ALU.is_gt)
                    val = work.tile([16, 64], F32, tag="val", bufs=2)
                    nc.vector.tensor_mul(val[:], mask[:], iota_f1[:])
                    nc.vector.tensor_scalar_add(val[:], val[:], -1.0)
                    ids_f = work.tile([16, CAP // 16], F32, tag="ids_f",
                                      bufs=2)
                    nf = work.tile([1, 1], U32, tag="nf", bufs=2)
                    nc.gpsimd.sparse_gather(ids_f[:], val[:],
                                            num_found=nf[:])
                    nc.sync.dma_start(nf_out.ap()[ei:ei + 1, :], nf[:])
                    nc.vector.tensor_scalar_min(ids_f[:], ids_f[:], 1023.0)
                    nc.vector.tensor_scalar_max(ids_f[:], ids_f[:], 0.0)
                    nc.sync.dma_start(ids_out.ap()[ei], ids_f[:])
                    ids_u = work.tile([16, CAP // 16], U16, tag="ids_u",
                                      bufs=2)
                    nc.vector.tensor_copy(out=ids_u[:], in_=ids_f[:])
                    idsr = work.tile([128, CAP // 16], U16, tag="idsr",
                                     bufs=2)
                    for g8 in range(8):
                        nc.sync.dma_start(idsr[g8 * 16:(g8 + 1) * 16, :],
                                          ids_u[:])
                    hg = []
                    for k in range(8):
                        t = work.tile([128, CAP], BF16, tag=f"hg{k}",
                                      bufs=1, name=f"hg{ei}_{k}")
                        nc.gpsimd.indirect_copy(t[:], ht_full[k][:],
                                                idsr[:], True)
                        hg.append(t)
                    # e1 = silu((hg^T W1)/64 + b1), feature-major
                    e1_bf = []
                    for colb in range(8):
                        w1t = []
                        for k in range(8):
                            wt = wstr.tile([128, 512], BF16, tag="wqf8",
                                           bufs=(16 if repeat == 1 else 8),
                                           name=f"w1_{ei}_{colb}_{k}")
                            nc.sync.dma_start(wt[:], w1.ap()[ei, colb, k])
                            w1t.append(wt)
                        for c4 in range(4):
                            col = colb * 4 + c4
                            pse = psC.tile([128, CAP], F32, tag="psmall",
                                           name=f"pse{ei}_{col}")
                            for k in range(8):
                                nc.tensor.matmul(
                                    pse[:],
                                    w1t[k][:, c4 * 128:(c4 + 1) * 128],
                                    hg[k][:], start=(k == 0),
                                    stop=(k == 7))
                            t = work.tile([128, CAP], BF16,
                                          tag=f"e1_{col}",
                                          name=f"e1_{ei}_{col}")
                            nc.scalar.activation(
                                t[:], pse[:], ACTF.Silu,
                                bias=b1_sb[ei][:, col:col + 1])
                            e1_bf.append(t)
                    # e2 = e1^T @ W2 (x64; host rescales)
                    psg = [psA.tile([128, 512], F32, tag="pnb",
                                    name=f"psg{ei}_{i}")
                           for i in range(2)] \
                        + [psB.tile([128, 512], F32, tag="pmid",
                                    name=f"psg{ei}_{i2 + 2}", bufs=2)
                           for i2 in range(2)]
                    for k2 in range(32):
                        wt = wstr.tile([128, D_], BF16, tag="w2s",
                                       bufs=(6 if repeat == 1 else 3),
                                       name=f"w2_{ei}_{k2}")
                        nc.sync.dma_start(wt[:], w2.ap()[ei, k2])
                        for tt_ in range(2):
                            nrow = 128 if tt_ == 0 else CAP - 128
                            for hf in range(2):
                                nc.tensor.matmul(
                                    psg[tt_ * 2 + hf][:nrow],
                                    e1_bf[k2][:, tt_ * 128:
                                              tt_ * 128 + nrow],
                                    wt[:, hf * 512:(hf + 1) * 512],
                                    start=(k2 == 0), stop=(k2 == 31))
                    for tt_ in range(2):
                        nrow = 128 if tt_ == 0 else CAP - 128
                        for hf in range(2):
                            cp = work.tile([128, 512], F32, tag="pcp",
                                           bufs=1,
                                           name=f"e2cp{ei}_{tt_}_{hf}")
                            nc.vector.tensor_copy(
                                out=cp[:nrow],
                                in_=psg[tt_ * 2 + hf][:nrow])
                            nc.sync.dma_start(
                                e2_out.ap()[ei,
                                            tt_ * 128:tt_ * 128 + nrow,
                                            hf * 512:(hf + 1) * 512],
                                cp[:nrow])

            for _rep in range(repeat):
                emit_body()

    nc.compile()
    return nc


# ---------------------------------------------------------------------------
def prep_inputs(inputs):
    f = {k: np.asarray(v, dtype=np.float32) for k, v in inputs.items()}
    xf = f["x"].reshape(TOK, D_)
    xT = np.ascontiguousarray(xf.T)
    perm = np.r_[np.arange(0, D_, 2), np.arange(1, D_, 2)]

    win_f = f["s6_Win"]
    bin_f = f["s6_bin"]
    WbF = win_f @ f["s6_WB"]
    WcF = win_f @ f["s6_WC"]
    WdF = win_f @ f["s6_Wd"]
    bBe = bin_f @ f["s6_WB"] + f["s6_bB"]
    bCe = bin_f @ f["s6_WC"] + f["s6_bC"]
    bde = bin_f @ f["s6_Wd"] + f["s6_bd"]

    wp_perm = np.concatenate([f["glcm_Wp"][:, :D_][:, perm],
                              f["glcm_Wp"][:, D_:][:, perm]], axis=1)
    bp_perm = np.concatenate([f["glcm_bp"][:D_][perm],
                              f["glcm_bp"][D_:][perm]])
    wdw_p = f["glcm_Wdw"][:, perm]
    bdw_p = f["glcm_bdw"][perm]
    wpb = np.zeros((8, 8, 128, 256), np.float32)
    for i in range(4):
        for k in range(8):
            wpb[i, k, :, :128] = wp_perm[k * 128:(k + 1) * 128,
                                         i * 128:(i + 1) * 128]
            wpb[i, k, :, 128:] = wp_perm[k * 128:(k + 1) * 128,
                                         512 + i * 128:512 + (i + 1) * 128]
    for j in range(4):
        for k in range(8):
            wpb[4 + j, k] = wp_perm[k * 128:(k + 1) * 128,
                                    D_ + j * 256:D_ + (j + 1) * 256]

    sinT = np.ascontiguousarray(f["sin"].T)
    cosT = np.ascontiguousarray(f["cos"].T)
    sin_pad = np.zeros((D_ // 2, T_ + 2 * HALO), np.float32)
    cos_pad = np.zeros((D_ // 2, T_ + 2 * HALO), np.float32)
    sin_pad[:, HALO:HALO + T_] = sinT
    cos_pad[:, HALO:HALO + T_] = cosT

    shared = {
        "xt_full": xT.astype(bf16),
        "wp_blk": wpb.astype(bf16),
        "bp_h": np.ascontiguousarray(bp_perm.reshape(16, 128).T),
        "wdw_h": np.ascontiguousarray(
            wdw_p.T.reshape(8, 128, K7).transpose(1, 0, 2).reshape(128, -1)),
        "bdw_h": np.ascontiguousarray(bdw_p.reshape(8, 128).T),
        "glcm_g": f["glcm_g"],
        "gfw1": f["gf_W1"].astype(bf16),
        "gfw2": f["gf_W2"].astype(bf16),
        "gfb1": f["gf_b1"], "gfb2": f["gf_b2"], "gf_g": f["gf_g"],
        "bout": f["s6_bout"], "s6g": f["s6_g"],
        "wg": f["moe_Wg"],
        "wout": f["s6_Wout"].astype(bf16),
    }

    A_full = -np.exp(f["s6_Alog"])

    in_maps = []
    for c in range(NC_):
        m = dict(shared)
        t0 = (c % 4) * TPC
        b0 = c // 4
        tok0 = c * TPC
        sc = slice(c * SPC, (c + 1) * SPC)

        seg = xT[:, b0 * T_:(b0 + 1) * T_]
        pad = np.zeros((D_, T_ + 2 * HALO), np.float32)
        pad[:, HALO:HALO + T_] = seg
        m["xt_halo"] = pad[:, t0:t0 + THW].astype(bf16)
        m["x_tm"] = np.ascontiguousarray(xf[tok0:tok0 + TPC])
        m["sin_t"] = np.ascontiguousarray(sin_pad[:, t0:t0 + THW])
        m["cos_t"] = np.ascontiguousarray(cos_pad[:, t0:t0 + THW])
        m["win_u"] = np.ascontiguousarray(win_f[:, sc]).astype(bf16)
        m["bin_u"] = np.ascontiguousarray(bin_f[sc][:, None])
        m["wdf"] = np.ascontiguousarray(WdF[:, sc]).astype(bf16)
        m["bd_h"] = np.ascontiguousarray(bde[sc][:, None])
        wb_r = WbF.reshape(D_, S_, N_)[:, sc, :].transpose(0, 2, 1)
        m["wbf"] = np.ascontiguousarray(
            wb_r.reshape(D_, 4, 512).transpose(1, 0, 2)
            .reshape(4, 8, 128, 512)).astype(bf16)
        wc_r = WcF.reshape(D_, S_, N_)[:, sc, :].transpose(0, 2, 1)
        m["wcf"] = np.ascontiguousarray(
            wc_r.reshape(D_, 4, 512).transpose(1, 0, 2)
            .reshape(4, 8, 128, 512)).astype(bf16)
        m["bb_h"] = np.ascontiguousarray(bBe.reshape(S_, N_)[sc])
        m["bc_h"] = np.ascontiguousarray(bCe.reshape(S_, N_)[sc])
        m["a_mat"] = np.ascontiguousarray(A_full[sc])
        ee = [2 * c, 2 * c + 1]
        w1s = f["moe_W1"][ee].reshape(2, 8, 128, 8, 512).transpose(
            0, 3, 1, 2, 4)
        m["w1"] = np.ascontiguousarray(w1s).astype(bf16)
        m["b1_h"] = np.ascontiguousarray(
            f["moe_b1"][ee].reshape(2, 32, 128).transpose(0, 2, 1))
        m["w2"] = np.ascontiguousarray(
            f["moe_W2"][ee].reshape(2, 32, 128, D_)).astype(bf16)
        in_maps.append(m)
    return in_maps


def postprocess(inputs, results):
    b2 = np.asarray(inputs["moe_b2"], dtype=np.float32)
    h_full = np.concatenate([r["h_out"] for r in results], axis=0)
    w_full = np.concatenate([r["w_out"] for r in results], axis=0)
    out = h_full.copy()
    for c in range(NC_):
        r = results[c]
        for ei in range(2):
            e = 2 * c + ei
            n = int(r["nf_out"][ei, 0])
            if n == 0:
                continue
            nd = min(n, CAP)
            ids_flat = r["ids_out"][ei].T.reshape(-1)[:nd].astype(np.int64)
            vals = r["e2_out"][ei, :nd] + b2[e][None, :]
            out[ids_flat] += w_full[ids_flat, e][:, None] * vals
            if n > CAP:
                # capacity overflow (~never): host computes the tail exactly
                sel = np.where(w_full[:, e] > 0)[0]
                missed = np.setdiff1d(sel, ids_flat)
                if len(missed):
                    W1 = np.asarray(inputs["moe_W1"][e], np.float32)
                    W2 = np.asarray(inputs["moe_W2"][e], np.float32)
                    b1v = np.asarray(inputs["moe_b1"][e], np.float32)
                    hh = h_full[missed]
                    a1 = hh @ W1 + b1v
                    a1 = a1 / (1.0 + np.exp(-a1))
                    ee2 = a1 @ W2 + b2[e]
                    out[missed] += w_full[missed, e][:, None] * ee2
    return out.reshape(B_, T_, D_)


_CACHE = {}


def kernel(**inputs):
    if "nc" not in _CACHE:
        _CACHE["nc"] = build_program()
    nc = _CACHE["nc"]
    in_maps = prep_inputs(inputs)
    res = bass_utils.run_bass_kernel_spmd(nc, in_maps,
                                          core_ids=list(range(NC_)))
    return postprocess(inputs, res.results)


if __name__ == "__main__":
    print("building...")
    build_program()
    print("built ok")
